# revision 16
# baseline (speedup 1.0000x reference)
"""DBN-Sigma whitening (group-wise decorrelated batch norm) on 8 trn2 cores.

Single-pass design (data-parallel over batch N, hint-conformant):
  The per-core shard (8 of 64 images) is cast to bf16 on the host and
  streamed ONCE into SBUF where it stays resident (98 KiB/partition).
  While loading, each (image-pair, channel-half) unit contributes
  per-channel row sums (ACT accum) and the two 128x128 diagonal blocks
  of the raw second moment, accumulated in PSUM via PE transposes +
  bf16 matmuls (software-pipelined in batches of 12/13 chunks so the
  DVE PSUM->SBUF bounce hides under the next batch's transposes).
  The stats compact to [128, 34] f32 (per-group 16x16 cov rows + row
  sums, 17 KB) and are AllGather'd across the 8 cores through DRAM
  bounce buffers, then summed on device.
  sigma = S2/M - mu mu^T + eps I (16x16 group blocks) is
  inverted-square-rooted ON DEVICE with 3 coupled Newton-Schulz
  iterations (f32 PE matmuls; sigma ~ I so NS converges quadratically).
  weight folds into the whiten stationary (wm diag(w), via per-partition
  scale + PE transpose); mean/bias fold into a per-channel shift, so
  PSUM evacuation is a single fused add spread over ACT/DVE/GpSimd.
  Whitening runs out of the resident SBUF copy (bf16 matmuls) and the
  bf16 output is upcast to f32 on the host.

HBM traffic per core: 12.85 MB in + 12.85 MB out (vs 64 MB for the
two-pass f32 baseline).
"""

import numpy as np
import ml_dtypes
import concourse.bass as bass
import concourse.bacc as bacc
import concourse.mybir as mybir
import concourse.tile as tile
from concourse.bass_utils import run_bass_kernel_spmd

N_CORES = 8
N, C, H, W = 64, 256, 56, 56
HW = H * W                     # 3136
NL = N // N_CORES              # 8 images per core
G, CG = 16, 16
EPS = 1e-3
M_TOT = N * HW                 # 200704
FP = mybir.dt.float32
BF = mybir.dt.bfloat16

NP_ = NL // 2                  # 4 image pairs per core
NU = 2 * NP_                   # 8 (pair, half) units
FPAIR = 2 * HW                 # 6272 free elems per (pair, half)
NCH = FPAIR // 128             # 49 m-chunks per unit
BATCHES = (12, 12, 12, 13)     # chunk batching for the transpose pipeline
KT = 448                       # whiten matmul free-dim tile (14 * 448 = 6272)
NKW = FPAIR // KT              # 14
NS_ITERS = 3
AF = mybir.ActivationFunctionType
ALU = mybir.AluOpType
# PSUM evacuation engine per whiten chunk (cycled): balance ACT/DVE/GpSimd
EVAC = ("act", "vec") * 7


def _build():
    nc = bacc.Bacc("TRN2", target_bir_lowering=False, debug=False,
                   num_devices=N_CORES)
    X_d = nc.dram_tensor("X", [NL, C, HW], BF, kind="ExternalInput")
    eyebf_d = nc.dram_tensor("eyebf", [128, 128], BF, kind="ExternalInput")
    eyef_d = nc.dram_tensor("eyef", [128, 128], FP, kind="ExternalInput")
    mask_d = nc.dram_tensor("mask", [128, 128], FP, kind="ExternalInput")
    eye15_d = nc.dram_tensor("eye15", [128, 128], FP, kind="ExternalInput")
    epsc_d = nc.dram_tensor("epsc", [128, 34], FP, kind="ExternalInput")
    w_d = nc.dram_tensor("wcol", [128, 2], FP, kind="ExternalInput")
    b_d = nc.dram_tensor("bcol", [128, 2], FP, kind="ExternalInput")
    Xn_d = nc.dram_tensor("Xn", [NL, C, HW], BF, kind="ExternalOutput")
    X = X_d.ap()
    Xn = Xn_d.ap()

    with tile.TileContext(nc) as tc:
        with (
            tc.tile_pool(name="const", bufs=1) as constp,
            tc.tile_pool(name="xres", bufs=1) as xresp,
            tc.tile_pool(name="scr", bufs=2) as scrp,
            tc.tile_pool(name="xtq", bufs=3) as xtqp,
            tc.tile_pool(name="stats", bufs=1) as statp,
            tc.tile_pool(name="small", bufs=1) as smallp,
            tc.tile_pool(name="out", bufs=3) as outp,
        ):
            eyebf = constp.tile([128, 128], BF, tag="eyebf")
            nc.sync.dma_start(eyebf[:], eyebf_d.ap())
            eyef = constp.tile([128, 128], FP, tag="eyef")
            nc.sync.dma_start(eyef[:], eyef_d.ap())
            mask = constp.tile([128, 128], FP, tag="mask")
            nc.sync.dma_start(mask[:], mask_d.ap())
            eye15 = constp.tile([128, 128], FP, tag="eye15")
            nc.sync.dma_start(eye15[:], eye15_d.ap())
            epsc = constp.tile([128, 34], FP, tag="epsc")
            nc.sync.dma_start(epsc[:], epsc_d.ap())
            wcol = constp.tile([128, 2], FP, tag="wcol")
            nc.sync.dma_start(wcol[:], w_d.ap())
            bcol = constp.tile([128, 2], FP, tag="bcol")
            nc.sync.dma_start(bcol[:], b_d.ap())

            xall = xresp.tile([128, NU, FPAIR], BF)       # resident shard
            rs = statp.tile([128, 2, NP_], FP)            # per-unit row sums
            cmp_sb = statp.tile([128, 34], FP)            # compact local stats
            gth = statp.tile([128, N_CORES, 34], FP)      # gathered stats
            red2 = statp.tile([128, 34], FP)              # summed / M + eps

            # ---- phase 1: load resident + stats -------------------------
            # software-pipelined PE program:  T(b) ; [T(b+1)] ; mm(b) ; ...
            with (
                tc.tile_pool(name="ptp", bufs=2, space="PSUM") as ptp,
                tc.tile_pool(name="covp", bufs=1, space="PSUM") as covp,
            ):
                cov = [covp.tile([128, 128], FP, tag=f"cov{h}", name=f"cov{h}")
                       for h in (0, 1)]
                started = [False, False]
                pending = None          # (h, xtq_tile, nch, last_of_cov)

                def flush_pending():
                    nonlocal pending
                    if pending is None:
                        return
                    fh, fx, fn, flast = pending
                    for jj in range(fn):
                        nc.tensor.matmul(
                            cov[fh][:], fx[:, 128 * jj:128 * (jj + 1)],
                            fx[:, 128 * jj:128 * (jj + 1)],
                            start=not started[fh],
                            stop=(flast and jj == fn - 1),
                            skip_group_check=True)
                        started[fh] = True
                    pending = None

                for p in range(NP_):
                    for h in (0, 1):
                        u = 2 * p + h
                        for i in (0, 1):
                            nc.sync.dma_start(
                                xall[:, u, HW * i:HW * (i + 1)],
                                X[2 * p + i, 128 * h:128 * (h + 1), :])
                        xu = xall[:, u, :]
                        scr = scrp.tile([128, FPAIR], BF, tag="scr")
                        nc.scalar.activation(scr[:], xu, AF.Copy,
                                             accum_out=rs[:, h, p:p + 1])
                        last_u = (p == NP_ - 1)
                        c0 = 0
                        for bi, nch in enumerate(BATCHES):
                            pt = ptp.tile([128, nch * 128], BF, tag="pt")
                            for jj in range(nch):
                                m0 = 128 * (c0 + jj)
                                nc.tensor.transpose(
                                    pt[:, 128 * jj:128 * (jj + 1)],
                                    xall[:, u, m0:m0 + 128], eyebf[:])
                            flush_pending()
                            xtq = xtqp.tile([128, nch * 128], BF, tag="xtq")
                            nc.vector.tensor_copy(xtq[:], pt[:])
                            pending = (h, xtq, nch,
                                       last_u and bi == len(BATCHES) - 1)
                            c0 += nch
                flush_pending()

                # compact: cmp[p, 16h+j] = sum_o (cov[h] * mask)[p, 16o+j]
                # (mask keeps only each row's own 16x16 block, so the sum
                #  over block-columns just picks out that block's entries)
                for h in (0, 1):
                    cm3 = statp.tile([128, 8, 16], FP, tag=f"cm3{h}",
                                     name=f"cm3{h}")
                    for o in range(8):
                        sl = slice(16 * o, 16 * (o + 1))
                        nc.vector.tensor_mul(cm3[:, o, :], cov[h][:, sl],
                                             mask[:, sl])
                    nc.vector.tensor_reduce(cmp_sb[:, 16 * h:16 * (h + 1)],
                                            cm3[:].transpose([0, 2, 1]),
                                            axis=mybir.AxisListType.X,
                                            op=ALU.add)
                    nc.vector.tensor_reduce(cmp_sb[:, 32 + h:33 + h],
                                            rs[:, h, :],
                                            axis=mybir.AxisListType.X,
                                            op=ALU.add)

            # ---- all-gather stats via direct SBUF->SBUF remote DMA ------
            # Sends are RELATIVE (Q7 XORs deltas with own ids): send j lands
            # my stats in slot j of core (me XOR j), so slot j on receiver r
            # holds core (r XOR j)'s stats -- a bijection, and we only sum.
            # Slot j for delta-tpb j satisfies the D2D slot rule (j&4 == j&4).
            rsem = nc.alloc_semaphore("gth_rsem")
            lsem = nc.alloc_semaphore("gth_lsem")
            nc.vector.tensor_copy(gth[:, 0, :], cmp_sb[:])   # self slot
            for j in range(1, N_CORES):
                rd = [None] * 8
                rd[j] = (0, j)
                nc.gpsimd.remote_dma_broadcast(gth[:, j, :], cmp_sb[:],
                                               rsem, lsem, rdests=rd)
            nc.gpsimd.trigger_dma(count=None)

            # red2 = (sum_cores stats) / M + eps-on-diag (compact layout)
            # (the remote-arrival gate rsem>=14 is attached to this reduce
            #  AFTER tile scheduling -- the single-core scheduling sim cannot
            #  model cross-core semaphore increments and would deadlock)
            red_inst = nc.vector.tensor_reduce(
                red2[:], gth[:].transpose([0, 2, 1]),
                axis=mybir.AxisListType.X, op=ALU.add)
            nc.vector.tensor_scalar(red2[:], red2[:], 1.0 / M_TOT, None,
                                    op0=ALU.mult)
            nc.vector.tensor_add(red2[:], red2[:], epsc[:])

            # ---- phase 2: sigma -> wm = sigma^(-1/2) on device ----------
            wmbf = smallp.tile([128, 256], BF, tag="wmbf")
            shift = smallp.tile([128, 2], FP, tag="shift")
            with tc.tile_pool(name="ps2", bufs=2, space="PSUM") as ps2p:
                meanc = red2[:, 32:34]    # already includes the /M
                Yt = [smallp.tile([128, 128], FP, tag=f"Y{h}", name=f"Y{h}")
                      for h in (0, 1)]
                Zt = [smallp.tile([128, 128], FP, tag=f"Z{h}", name=f"Z{h}")
                      for h in (0, 1)]
                Tt = [smallp.tile([128, 128], FP, tag=f"T{h}", name=f"T{h}")
                      for h in (0, 1)]
                for h in (0, 1):
                    # mean row (for the outer product) via PE transpose
                    s1pad = smallp.tile([128, 128], FP, tag=f"s1pad{h}",
                                        name=f"s1pad{h}")
                    nc.vector.memset(s1pad[:], 0.0)
                    nc.vector.tensor_copy(s1pad[:, 0:1], meanc[:, h:h + 1])
                    psT = ps2p.tile([128, 128], FP, tag="ps2s")
                    nc.tensor.transpose(psT[:], s1pad[:], eyef[:])
                    mT = smallp.tile([128, 128], FP, tag=f"meanT{h}",
                                     name=f"meanT{h}")
                    nc.scalar.activation(mT[0:1, :], psT[0:1, :], AF.Copy)
                    # sigma: expand compact blocks (stripe-wise mask mult),
                    # then subtract the masked mu mu^T outer product
                    for o in range(8):
                        sl = slice(16 * o, 16 * (o + 1))
                        eng = nc.gpsimd if o % 2 == 0 else nc.vector
                        eng.tensor_mul(Yt[h][:, sl],
                                       red2[:, 16 * h:16 * (h + 1)],
                                       mask[:, sl])
                    po = ps2p.tile([128, 128], FP, tag="ps2s")
                    nc.tensor.matmul(po[:], mT[0:1, :], mT[0:1, :],
                                     start=True, stop=True)
                    pom = smallp.tile([128, 128], FP, tag=f"pom{h}",
                                      name=f"pom{h}")
                    nc.vector.tensor_mul(pom[:], po[:], mask[:])
                    nc.vector.tensor_sub(Yt[h][:], Yt[h][:], pom[:])
                    nc.gpsimd.tensor_copy(Zt[h][:], eyef[:])
                # Newton-Schulz: T = 1.5 I - 0.5 Z Y ; Y <- Y T ; Z <- T Z
                # (iterates are polynomials in sigma -> symmetric, so
                #  matmul's lhsT-transpose is a no-op)
                for it in range(NS_ITERS):
                    for h in (0, 1):
                        pzy = ps2p.tile([128, 128], FP, tag="ps2s")
                        nc.tensor.matmul(pzy[:], Zt[h][:], Yt[h][:],
                                         start=True, stop=True)
                        nc.vector.scalar_tensor_tensor(
                            Tt[h][:], pzy[:], -0.5, eye15[:],
                            op0=ALU.mult, op1=ALU.add)
                        if it < NS_ITERS - 1:
                            pyy = ps2p.tile([128, 128], FP, tag="ps2s")
                            nc.tensor.matmul(pyy[:], Yt[h][:], Tt[h][:],
                                             start=True, stop=True)
                            nc.scalar.activation(Yt[h][:], pyy[:], AF.Copy)
                        pzz = ps2p.tile([128, 128], FP, tag="ps2s")
                        nc.tensor.matmul(pzz[:], Tt[h][:], Zt[h][:],
                                         start=True, stop=True)
                        nc.vector.tensor_copy(Zt[h][:], pzz[:])
                # fold weight into the stationary: lhsT = (diag(w) wm)^T
                for h in (0, 1):
                    wmw = smallp.tile([128, 128], FP, tag=f"wmw{h}",
                                      name=f"wmw{h}")
                    nc.vector.tensor_scalar(wmw[:], Zt[h][:],
                                            wcol[:, h:h + 1], None,
                                            op0=ALU.mult)
                    pwT = ps2p.tile([128, 128], FP, tag="ps2s")
                    nc.tensor.transpose(pwT[:], wmw[:], eyef[:])
                    nc.vector.tensor_copy(wmbf[:, 128 * h:128 * (h + 1)],
                                          pwT[:])
                    pm = ps2p.tile([128, 1], FP, tag="pm")
                    nc.tensor.matmul(pm[:], Zt[h][:], meanc[:, h:h + 1],
                                     start=True, stop=True)
                    nc.vector.tensor_mul(shift[:, h:h + 1], wcol[:, h:h + 1],
                                        pm[:])
                    nc.vector.tensor_sub(shift[:, h:h + 1], bcol[:, h:h + 1],
                                        shift[:, h:h + 1])

            # ---- phase 3: whiten from resident SBUF ---------------------
            with tc.tile_pool(name="psw", bufs=6, space="PSUM") as pswp:
                for p in range(NP_):
                    for h in (0, 1):
                        u = 2 * p + h
                        ot = outp.tile([128, FPAIR], BF, tag="ot")
                        for k in range(NKW):
                            ps = pswp.tile([128, KT], FP, tag="psw")
                            nc.tensor.matmul(
                                ps[:], wmbf[:, 128 * h:128 * (h + 1)],
                                xall[:, u, KT * k:KT * (k + 1)],
                                start=True, stop=True)
                            osl = ot[:, KT * k:KT * (k + 1)]
                            eng = EVAC[k]
                            if eng == "act":
                                nc.scalar.activation(
                                    osl, ps[:], AF.Identity,
                                    bias=shift[:, h:h + 1], scale=1.0)
                            else:
                                nc.vector.tensor_scalar(
                                    osl, ps[:], shift[:, h:h + 1], None,
                                    op0=ALU.add)
                        for i in (0, 1):
                            nc.sync.dma_start(
                                Xn[2 * p + i, 128 * h:128 * (h + 1), :],
                                ot[:, HW * i:HW * (i + 1)])

    import bass_rust
    si = red_inst.ins.sync_info
    ow = si.on_wait
    ow.append(bass_rust.SyncWait(
        sync_type="semaphore", id=rsem.num, ant_name=rsem.name,
        wait_mode="sem-ge-imm", wait_value=2 * (N_CORES - 1), wait_reg=None))
    si.on_wait = ow

    nc.compile()
    return nc


_PROGS = {}


def _program():
    if "k" not in _PROGS:
        _PROGS["k"] = _build()
    return _PROGS["k"]


def _const_inputs(weight, bias):
    eyebf = np.eye(128, dtype=ml_dtypes.bfloat16)
    eyef = np.eye(128, dtype=np.float32)
    mask = np.kron(np.eye(8, dtype=np.float32),
                   np.ones((CG, CG), dtype=np.float32))
    eye15 = (1.5 * np.eye(128)).astype(np.float32)
    epsc = np.zeros((128, 34), dtype=np.float32)
    for p in range(128):
        epsc[p, p % 16] = EPS
        epsc[p, 16 + p % 16] = EPS
    wcol = np.ascontiguousarray(weight.reshape(2, 128).T.astype(np.float32))
    bcol = np.ascontiguousarray(bias.reshape(2, 128).T.astype(np.float32))
    return {"eyebf": eyebf, "eyef": eyef, "mask": mask, "eye15": eye15,
            "epsc": epsc, "wcol": wcol, "bcol": bcol}


def kernel(X, weight, bias, _return_results=False):
    X = np.asarray(X, dtype=np.float32)
    weight = np.asarray(weight, dtype=np.float32).reshape(C)
    bias = np.asarray(bias, dtype=np.float32).reshape(C)
    nc = _program()

    Xr = X.reshape(N, C, HW)
    consts = _const_inputs(weight, bias)
    in_maps = [{"X": Xr[NL * i:NL * (i + 1)].astype(ml_dtypes.bfloat16),
                **consts} for i in range(N_CORES)]

    res = run_bass_kernel_spmd(nc, in_maps, list(range(N_CORES)))

    out = np.concatenate([r["Xn"].astype(np.float32) for r in res.results],
                         axis=0)
    out = out.reshape(N, C, H, W)
    if _return_results:
        return out, res
    return out
